# revision 3
# baseline (speedup 1.0000x reference)
"""Trainium2 Bass kernel for a binarized-conv BasicBlock (dense_cnn).

Computation (matches the reference nn.Module):
    out = clip(BN2(conv3x3(binarize(clip(BN1(conv3x3(binarize(x), binarize(w1))))),
                  binarize(w2)) + x))
with training-mode (batch-stats) BN over the full 64-image batch.

Strategy:
  - Data-parallel over batch: 8 images per core on 8 NeuronCores.
  - Binarized 3x3 conv as 9 accumulating DoubleRow fp8 PE matmuls (K=256)
    per [128, 392] output tile over zero-padded [128, 2, 30, 32] fp8
    activation tiles; +-1/+-0.5 values in fp8 are exact, PSUM is fp32.
  - BN1 + hardtanh + binarize collapses to a per-channel threshold compare;
    sync-BN via a single [128, 4] AllReduce, its latency bridged by the w2
    weight-prep transposes on the otherwise-idle PE.
  - conv2 runs output-block-major so BN2's sync splits per block: AR2a and
    the whole ob0 output phase (affine+clamp+DMA) hide under conv2-ob1's PE
    time; only AR2b plus the ob1 output burst remain in the tail.
  - Padded activation buffers get border-only memsets and are recycled
    between conv1 and conv2 inputs (interior is always fully rewritten).
"""

import os
import sys

import numpy as np


def _ensure_paths():
    for p in ("/opt/trn_rl_repo", "/root/.axon_site/_ro/trn_rl_repo"):
        if p not in sys.path and os.path.isdir(p):
            sys.path.append(p)


try:
    from concourse import bacc, mybir, tile  # noqa: F401
except ImportError:
    _ensure_paths()
    from concourse import bacc, mybir, tile  # noqa: F401

from concourse.bass_utils import run_bass_kernel_spmd
from concourse.masks import make_identity

N_CORES = 8
IMGS = 8          # images per core (64 / 8)
C = 256
CB = 2            # channel blocks of 128
H = W = 28
HP = WP = 30      # zero-padded spatial
PIX = H * W       # 784
HALF = PIX // 2   # 392 (one PSUM bank of fp32)
NT = 64 * PIX     # BN count over the GLOBAL batch (N*H*W)
EPS = 1e-5
NPAD = IMGS + 2   # physical padded-activation buffers (recycled)

F32 = mybir.dt.float32
BF16 = mybir.dt.bfloat16
FP8 = mybir.dt.float8e4
AF = mybir.ActivationFunctionType
ALU = mybir.AluOpType
DR = mybir.MatmulPerfMode.DoubleRow

# padded fp8 activation layout: [128, 2 kblocks, 30 rows, 32 cols]
RP = 32           # row pitch (28 cols + pad, %16 bytes)
KP = HP * RP      # per-kblock pitch = 960

_PROGRAM = None


def _build_program():
    nc = bacc.Bacc("TRN2", target_bir_lowering=False, debug=False,
                   num_devices=N_CORES)

    x_in = nc.dram_tensor("x", [IMGS, C, H, W], F32, kind="ExternalInput").ap()
    w1_in = nc.dram_tensor("w1", [C, C, 3, 3], F32, kind="ExternalInput").ap()
    w2_in = nc.dram_tensor("w2", [C, C, 3, 3], F32, kind="ExternalInput").ap()
    g1_in = nc.dram_tensor("gamma1", [C], F32, kind="ExternalInput").ap()
    b1_in = nc.dram_tensor("beta1", [C], F32, kind="ExternalInput").ap()
    g2_in = nc.dram_tensor("gamma2", [C], F32, kind="ExternalInput").ap()
    b2_in = nc.dram_tensor("beta2", [C], F32, kind="ExternalInput").ap()
    out_d = nc.dram_tensor("out", [IMGS, C, H, W], F32, kind="ExternalOutput").ap()

    groups = [list(range(N_CORES))]

    with tile.TileContext(nc) as tc:
        with (
            tc.tile_pool(name="consts", bufs=1) as p_const,
            tc.tile_pool(name="wstage", bufs=3) as p_wstage,
            tc.tile_pool(name="wt", bufs=2 * 9 * 2) as p_wt,
            tc.tile_pool(name="xp", bufs=IMGS * CB) as p_x,
            tc.tile_pool(name="apad", bufs=NPAD) as p_apad,
            tc.tile_pool(name="yz", bufs=IMGS * CB) as p_yz,
            tc.tile_pool(name="sq", bufs=2) as p_sq,
            tc.tile_pool(name="o1", bufs=6) as p_o1,
            tc.tile_pool(name="ps", bufs=8, space="PSUM") as p_ps,
            tc.tile_pool(name="dram", bufs=1, space="DRAM") as p_dram,
        ):
            # Warmup collective: absorbs the ~11us ncfw wake + first-mesh-op
            # overhead on stream 0 while conv1 runs, so the BN sync
            # AllReduces behave like warm ops (~10us instead of ~38us).
            ccw_i = p_dram.tile([128, 1], F32, name="ccw_i")
            ccw_o = p_dram.tile([128, 1], F32, name="ccw_o")
            zz = p_const.tile([128, 1], F32, name="zz")
            nc.vector.memset(zz, 0.0)
            nc.sync.dma_start(out=ccw_i, in_=zz)
            nc.gpsimd.collective_compute(
                "AllReduce", ALU.add, replica_groups=groups,
                ins=[ccw_i.opt()], outs=[ccw_o.opt()])

            ident = p_const.tile([128, 128], F32, name="ident")
            make_identity(nc, ident)

            def dma_chunked(out_ap, in_ap, parts, engs):
                """Split a big DMA along the last free dim so the transfer
                spreads across several DMA engines; rotate the issuing
                queue over `engs` so the ~0.6us per-issue cost is
                parallelized across sequencers."""
                n = out_ap.shape[-1]
                step = (n + parts - 1) // parts
                for ci, a in enumerate(range(0, n, step)):
                    b = min(a + step, n)
                    engs[ci % len(engs)].dma_start(
                        out=out_ap[:, a:b], in_=in_ap[:, a:b])

            # gamma/beta as [128, 2] (col = channel block)
            def load_cvec(src, nm):
                t = p_const.tile([128, CB], F32, name=nm)
                nc.sync.dma_start(out=t, in_=src.rearrange("(b p) -> p b", p=128))
                return t

            g1t = load_cvec(g1_in, "g1t")
            b1t = load_cvec(b1_in, "b1t")
            rg1 = p_const.tile([128, CB], F32, name="rg1")
            nc.vector.reciprocal(rg1, g1t)
            bg1 = p_const.tile([128, CB], F32, name="bg1")
            nc.vector.tensor_mul(bg1, b1t, rg1)
            g2t = load_cvec(g2_in, "g2t")
            b2t = load_cvec(b2_in, "b2t")

            # per-channel stat accumulators, one column per (img, half)
            def stat_tiles(nm):
                return [p_const.tile([128, IMGS * 2], F32, name=f"{nm}{ob}")
                        for ob in range(CB)]

            st1s, st1q = stat_tiles("st1s"), stat_tiles("st1q")
            st2s, st2q = stat_tiles("st2s"), stat_tiles("st2q")

            # ---- padded fp8 activation buffers: border-only memsets ----
            # The binarize writes always cover [1:29, 1:29]; only the halo
            # (row 0, row 29, col 0, cols 29-31) must be zero, and it stays
            # zero when a buffer is recycled between conv1 and conv2 inputs.
            pad = []
            for i in range(NPAD):
                t = p_apad.tile([128, CB * KP], FP8, tag="apad",
                                name=f"pad{i}")
                pad.append(t)
                a4 = t.rearrange("p (k r c) -> p k r c", k=CB, r=HP)
                for b in range(CB):
                    nc.gpsimd.memset(a4[:, b, 0, :], 0.0)
                    nc.gpsimd.memset(a4[:, b, 29, :], 0.0)
                    nc.gpsimd.memset(a4[:, b, 1:29, 0:1], 0.0)
                    nc.gpsimd.memset(a4[:, b, 1:29, 29:32], 0.0)

            xsign = [pad[n] for n in range(IMGS)]
            b2a = [pad[(IMGS + n) % NPAD] for n in range(IMGS)]

            # ---- weight staging (DMA) and prep (PE transpose + Sign) ----
            def stage_w(w_in, wi, ob):
                wst = p_wstage.tile([128, C * 9], F32, tag="wst",
                                    name=f"wst{wi}_{ob}")
                dma_chunked(
                    wst,
                    w_in[ob * 128:(ob + 1) * 128].rearrange(
                        "o i ky kx -> o (i ky kx)"),
                    parts=6, engs=[nc.sync, nc.gpsimd])
                return wst

            def prep_w(wst, wi, ob, wt):
                w3 = wst.rearrange("p (i t) -> p i t", t=9)
                for tap in range(9):
                    t = p_wt.tile([128, CB * 128], FP8, tag="wt",
                                  name=f"wt{wi}_{tap}_{ob}")
                    wt[(tap, ob)] = t
                    for kb in range(CB):
                        ps = p_ps.tile([128, 128], F32, tag="ps",
                                       name=f"pst{wi}_{ob}_{kb}_{tap}")
                        nc.tensor.transpose(
                            ps, w3[:, kb * 128:(kb + 1) * 128, tap], ident)
                        nc.scalar.activation(
                            t[:, kb * 128:(kb + 1) * 128], ps, AF.Sign)

            # ---- x: load raw f32 (kept for the residual), sign -> fp8 ----
            xt = [[None] * CB for _ in range(IMGS)]

            def load_x(n):
                a4 = xsign[n].rearrange("p (k r c) -> p k r c", k=CB, r=HP)
                for b in range(CB):
                    xr = p_x.tile([128, PIX], F32, tag="xp", name=f"x_{n}_{b}")
                    dma_chunked(
                        xr,
                        x_in[n, b * 128:(b + 1) * 128].rearrange(
                            "c h w -> c (h w)"),
                        parts=2, engs=[nc.sync, nc.gpsimd])
                    xt[n][b] = xr
                    nc.vector.tensor_scalar(
                        out=a4[:, b, 1:29, 1:29],
                        in0=xr.rearrange("p (h w) -> p h w", h=H),
                        scalar1=0.0, scalar2=0.5,
                        op0=ALU.is_ge, op1=ALU.subtract)

            # head: w1-ob0 staging leads, then the first image pair, then
            # w1-ob1 (needed at ~pair0-ob1), then the x stream, then w2.
            wt1, wt2 = {}, {}
            ws10 = stage_w(w1_in, 1, 0)
            load_x(0)
            load_x(1)
            ws11 = stage_w(w1_in, 1, 1)
            prep_w(ws10, 1, 0, wt1)
            for n in range(2, IMGS):
                load_x(n)
            ws20 = stage_w(w2_in, 2, 0)
            ws21 = stage_w(w2_in, 2, 1)

            # ---- conv: 9 DoubleRow matmuls (K=256) per [128, 392] PSUM ----
            def emit_group(wt, act, evict, pair, ob):
                tiles = [(n, half)
                         for n in (2 * pair, 2 * pair + 1)
                         for half in range(2)]
                pss = {}
                for (n, half) in tiles:
                    pss[(n, half)] = p_ps.tile(
                        [128, HALF], F32, tag="ps",
                        name=f"ps_{ob}_{n}_{half}")
                for tap in range(9):
                    dy, dx = divmod(tap, 3)
                    w3 = wt[(tap, ob)].rearrange(
                        "p (k o) -> p k o", k=CB)
                    for (n, half) in tiles:
                        a4 = act[n].rearrange(
                            "p (k r c) -> p k r c", k=CB, r=HP)
                        rhs = a4[:, :, dy + half * 14: dy + half * 14 + 14,
                                 dx: dx + W]
                        nc.tensor.matmul(pss[(n, half)], w3, rhs,
                                         start=(tap == 0),
                                         stop=(tap == 8),
                                         perf_mode=DR)
                for (n, half) in tiles:
                    evict(n, ob, half, pss[(n, half)])

            # ---- conv1 eviction: copy PSUM->y1 with sum, square w/ sumsq ----
            y1 = [[None] * CB for _ in range(IMGS)]

            def evict1(n, ob, half, ps):
                if y1[n][ob] is None:
                    y1[n][ob] = p_yz.tile([128, PIX], F32, tag="yz",
                                          name=f"y1_{n}_{ob}")
                idx = n * 2 + half
                ysl = y1[n][ob][:, half * HALF:(half + 1) * HALF]
                nc.scalar.activation(ysl, ps, AF.Copy, scale=2.0,
                                     accum_out=st1s[ob][:, idx:idx + 1])
                sq = p_sq.tile([128, HALF], F32, tag="sq")
                nc.vector.scalar_tensor_tensor(
                    out=sq, in0=ysl, scalar=1.0, in1=ysl,
                    op0=ALU.mult, op1=ALU.mult,
                    accum_out=st1q[ob][:, idx:idx + 1])

            # conv1: image-major (pairs outer) relaxes the x-load deadlines;
            # w1-ob1 prep slots between the first two groups.
            emit_group(wt1, xsign, evict1, 0, 0)
            prep_w(ws11, 1, 1, wt1)
            emit_group(wt1, xsign, evict1, 0, 1)
            for pair in range(1, IMGS // 2):
                for ob in range(CB):
                    emit_group(wt1, xsign, evict1, pair, ob)

            # ---- BN1: AllReduce global sums, derive per-channel thresholds ----
            pk1 = p_const.tile([128, 2 * CB], F32, name="pk1")
            for ob in range(CB):
                nc.vector.tensor_reduce(out=pk1[:, 2 * ob:2 * ob + 1],
                                        in_=st1s[ob], axis=mybir.AxisListType.X,
                                        op=ALU.add)
                nc.vector.tensor_reduce(out=pk1[:, 2 * ob + 1:2 * ob + 2],
                                        in_=st1q[ob], axis=mybir.AxisListType.X,
                                        op=ALU.add)
            cc1i = p_dram.tile([128, 2 * CB], F32, name="cc1i")
            cc1o = p_dram.tile([128, 2 * CB], F32, name="cc1o")
            nc.sync.dma_start(out=cc1i, in_=pk1)
            nc.gpsimd.collective_compute(
                "AllReduce", ALU.add, replica_groups=groups,
                ins=[cc1i.opt()], outs=[cc1o.opt()])
            red1 = p_const.tile([128, 2 * CB], F32, name="red1")
            nc.sync.dma_start(out=red1, in_=cc1o)
            r3 = red1.rearrange("p (b k) -> p b k", k=2)

            # w2 prep fills the sync-BN wait on the PE
            prep_w(ws20, 2, 0, wt2)
            prep_w(ws21, 2, 1, wt2)

            m1 = p_const.tile([128, CB], F32, name="m1")
            nc.vector.tensor_scalar(out=m1, in0=r3[:, :, 0], scalar1=1.0 / NT,
                                    scalar2=None, op0=ALU.mult)
            e1 = p_const.tile([128, CB], F32, name="e1")
            nc.vector.tensor_scalar(out=e1, in0=r3[:, :, 1], scalar1=1.0 / NT,
                                    scalar2=None, op0=ALU.mult)
            mm1 = p_const.tile([128, CB], F32, name="mm1")
            nc.vector.tensor_mul(mm1, m1, m1)
            v1 = p_const.tile([128, CB], F32, name="v1")
            nc.vector.tensor_sub(v1, e1, mm1)
            v1e = p_const.tile([128, CB], F32, name="v1e")
            nc.vector.tensor_scalar(out=v1e, in0=v1, scalar1=EPS, scalar2=None,
                                    op0=ALU.add)
            sd1 = p_const.tile([128, CB], F32, name="sd1")
            nc.scalar.activation(sd1, v1e, AF.Sqrt)
            tb1 = p_const.tile([128, CB], F32, name="tb1")
            nc.vector.tensor_mul(tb1, bg1, sd1)
            thr1 = p_const.tile([128, CB], F32, name="thr1")
            nc.vector.tensor_sub(thr1, m1, tb1)

            # ---- binarize(BN1(y1)) == is_ge(y1, thr) - 0.5 (padded fp8) ----
            for n in range(IMGS):
                a4 = b2a[n].rearrange("p (k r c) -> p k r c", k=CB, r=HP)
                for b in range(CB):
                    nc.vector.tensor_scalar(
                        out=a4[:, b, 1:29, 1:29],
                        in0=y1[n][b].rearrange("p (h w) -> p h w", h=H),
                        scalar1=thr1[:, b:b + 1], scalar2=0.5,
                        op0=ALU.is_ge, op1=ALU.subtract)

            # ---- conv2 eviction: z = 2*psum + x (fused sum), square ----
            z = [[None] * CB for _ in range(IMGS)]

            def evict2(n, ob, half, ps):
                if z[n][ob] is None:
                    z[n][ob] = p_yz.tile([128, PIX], F32, tag="yz",
                                         name=f"z_{n}_{ob}")
                idx = n * 2 + half
                zsl = z[n][ob][:, half * HALF:(half + 1) * HALF]
                nc.vector.scalar_tensor_tensor(
                    out=zsl, in0=ps, scalar=2.0,
                    in1=xt[n][ob][:, half * HALF:(half + 1) * HALF],
                    op0=ALU.mult, op1=ALU.add,
                    accum_out=st2s[ob][:, idx:idx + 1])
                sq = p_sq.tile([128, HALF], F32, tag="sq")
                nc.scalar.activation(sq, zsl, AF.Square,
                                     accum_out=st2q[ob][:, idx:idx + 1])

            # ---- BN2 per output block: reduce, AllReduce, affine params ----
            def bn2_sync(ob):
                pk = p_const.tile([128, 2], F32, name=f"pk2_{ob}")
                nc.vector.tensor_reduce(out=pk[:, 0:1], in_=st2s[ob],
                                        axis=mybir.AxisListType.X, op=ALU.add)
                nc.vector.tensor_reduce(out=pk[:, 1:2], in_=st2q[ob],
                                        axis=mybir.AxisListType.X, op=ALU.add)
                cci = p_dram.tile([128, 2], F32, name=f"cc2i_{ob}")
                cco = p_dram.tile([128, 2], F32, name=f"cc2o_{ob}")
                nc.sync.dma_start(out=cci, in_=pk)
                nc.gpsimd.collective_compute(
                    "AllReduce", ALU.add, replica_groups=groups,
                    ins=[cci.opt()], outs=[cco.opt()])
                red = p_const.tile([128, 2], F32, name=f"red2_{ob}")
                nc.sync.dma_start(out=red, in_=cco)
                m2 = p_const.tile([128, 1], F32, name=f"m2_{ob}")
                nc.vector.tensor_scalar(out=m2, in0=red[:, 0:1],
                                        scalar1=1.0 / NT, scalar2=None,
                                        op0=ALU.mult)
                e2 = p_const.tile([128, 1], F32, name=f"e2_{ob}")
                nc.vector.tensor_scalar(out=e2, in0=red[:, 1:2],
                                        scalar1=1.0 / NT, scalar2=None,
                                        op0=ALU.mult)
                mm2 = p_const.tile([128, 1], F32, name=f"mm2_{ob}")
                nc.vector.tensor_mul(mm2, m2, m2)
                v2 = p_const.tile([128, 1], F32, name=f"v2_{ob}")
                nc.vector.tensor_sub(v2, e2, mm2)
                v2f = p_const.tile([128, 1], F32, name=f"v2f_{ob}")
                nc.vector.tensor_scalar(out=v2f, in0=v2, scalar1=EPS,
                                        scalar2=None, op0=ALU.add)
                rc2 = p_const.tile([128, 1], F32, name=f"rc2_{ob}")
                nc.vector.reciprocal(rc2, v2f)
                rstd = p_const.tile([128, 1], F32, name=f"rstd_{ob}")
                nc.scalar.activation(rstd, rc2, AF.Sqrt)
                fs = p_const.tile([128, 1], F32, name=f"fs_{ob}")
                nc.vector.tensor_mul(fs, g2t[:, ob:ob + 1], rstd)
                msc = p_const.tile([128, 1], F32, name=f"msc_{ob}")
                nc.vector.tensor_mul(msc, m2, fs)
                fb = p_const.tile([128, 1], F32, name=f"fb_{ob}")
                nc.vector.tensor_sub(fb, b2t[:, ob:ob + 1], msc)
                return fs, fb

            # ---- final: clip(z * fscale + fbias) -> DRAM ----
            def out_tile(n, ob, fs, fb, aff_eng, clamp_eng, dma_engs):
                o1 = p_o1.tile([128, PIX], F32, tag="o1")
                if aff_eng is nc.scalar:
                    nc.scalar.activation(o1, z[n][ob], AF.Identity,
                                         bias=fb, scale=fs)
                else:
                    aff_eng.tensor_scalar(
                        out=o1, in0=z[n][ob], scalar1=fs, scalar2=fb,
                        op0=ALU.mult, op1=ALU.add)
                clamp_eng.tensor_scalar(out=o1, in0=o1, scalar1=-1.0,
                                        scalar2=1.0, op0=ALU.max, op1=ALU.min)
                dma_chunked(
                    out_d[n, ob * 128:(ob + 1) * 128].rearrange(
                        "c h w -> c (h w)"),
                    o1, parts=2, engs=dma_engs)

            # conv2-ob0, then its BN sync; the ob0 output phase rides on
            # GpSimd + the sync DMA queue underneath conv2-ob1's PE time.
            for pair in range(IMGS // 2):
                emit_group(wt2, b2a, evict2, pair, 0)
            fs0, fb0 = bn2_sync(0)

            emit_group(wt2, b2a, evict2, 0, 1)
            emit_group(wt2, b2a, evict2, 1, 1)
            out_tile(0, 0, fs0, fb0, nc.gpsimd, nc.gpsimd, [nc.sync])
            out_tile(1, 0, fs0, fb0, nc.gpsimd, nc.gpsimd, [nc.sync])
            emit_group(wt2, b2a, evict2, 2, 1)
            out_tile(2, 0, fs0, fb0, nc.gpsimd, nc.gpsimd, [nc.sync])
            out_tile(3, 0, fs0, fb0, nc.gpsimd, nc.gpsimd, [nc.sync])
            emit_group(wt2, b2a, evict2, 3, 1)
            for n in range(4, IMGS):
                out_tile(n, 0, fs0, fb0, nc.gpsimd, nc.gpsimd, [nc.sync])

            fs1, fb1 = bn2_sync(1)
            for n in range(IMGS):
                aff = nc.scalar if n % 2 == 0 else nc.vector
                clamp = nc.vector if n % 2 == 0 else nc.gpsimd
                dmae = [[nc.sync, nc.gpsimd], [nc.scalar, nc.sync]][n % 2]
                out_tile(n, 1, fs1, fb1, aff, clamp, dmae)

    nc.compile()
    return nc


def _get_program():
    global _PROGRAM
    if _PROGRAM is None:
        _PROGRAM = _build_program()
    return _PROGRAM


def run_sharded(inputs, **spmd_kwargs):
    """Shard inputs across 8 cores, run, and gather. Returns (out, results)."""
    nc = _get_program()
    x = np.ascontiguousarray(np.asarray(inputs["x"], dtype=np.float32))
    base = {
        k: np.ascontiguousarray(np.asarray(inputs[k], dtype=np.float32))
        for k in ("w1", "w2", "gamma1", "beta1", "gamma2", "beta2")
    }
    shards = np.split(x, N_CORES, axis=0)
    in_maps = [{"x": shards[i], **base} for i in range(N_CORES)]
    res = run_bass_kernel_spmd(nc, in_maps, core_ids=list(range(N_CORES)),
                               **spmd_kwargs)
    out = np.concatenate([res.results[i]["out"] for i in range(N_CORES)],
                         axis=0).astype(np.float32)
    return out, res


def kernel(**inputs):
    out, _ = run_sharded(inputs)
    return out


# revision 11
# speedup vs baseline: 1.6745x; 1.6745x over previous
"""Trainium2 Bass kernel for a binarized-conv BasicBlock (dense_cnn).

Computation (matches the reference nn.Module):
    out = clip(BN2(conv3x3(binarize(clip(BN1(conv3x3(binarize(x), binarize(w1))))),
                  binarize(w2)) + x))
with training-mode (batch-stats) BN over the full 64-image batch.

Strategy:
  - Data-parallel over batch: 8 images per core on 8 NeuronCores.
  - Binarized 3x3 conv as 9 accumulating DoubleRow fp8 PE matmuls (K=256)
    per [128, 392] output tile over zero-padded [128, 2, 30, 32] fp8
    activation tiles; +-1/+-0.5 values in fp8 are exact, PSUM is fp32.
  - BN1 + hardtanh + binarize collapses to a per-channel threshold compare;
    sync-BN via a single [128, 4] AllReduce, its latency bridged by the w2
    weight-prep transposes on the otherwise-idle PE.
  - conv2 runs output-block-major so BN2's sync splits per block: AR2a and
    the whole ob0 output phase (affine+clamp+DMA) hide under conv2-ob1's PE
    time; only AR2b plus the ob1 output burst remain in the tail.
  - Padded activation buffers get border-only memsets and are recycled
    between conv1 and conv2 inputs (interior is always fully rewritten).
"""

import os
import sys

import numpy as np


def _ensure_paths():
    for p in ("/opt/trn_rl_repo", "/root/.axon_site/_ro/trn_rl_repo"):
        if p not in sys.path and os.path.isdir(p):
            sys.path.append(p)


try:
    from concourse import bacc, mybir, tile  # noqa: F401
except ImportError:
    _ensure_paths()
    from concourse import bacc, mybir, tile  # noqa: F401

from concourse.bass_utils import run_bass_kernel_spmd
from concourse.masks import make_identity

N_CORES = 8
IMGS = 8          # images per core (64 / 8)
C = 256
CB = 2            # channel blocks of 128
H = W = 28
HP = WP = 30      # zero-padded spatial
PIX = H * W       # 784
HALF = PIX // 2   # 392 (one PSUM bank of fp32)
NT = 64 * PIX     # BN count over the GLOBAL batch (N*H*W)
EPS = 1e-5
NPAD = IMGS + 2   # physical padded-activation buffers (recycled)

F32 = mybir.dt.float32
BF16 = mybir.dt.bfloat16
FP8 = mybir.dt.float8e4
AF = mybir.ActivationFunctionType
ALU = mybir.AluOpType
DR = mybir.MatmulPerfMode.DoubleRow

# padded fp8 activation layout: [128, 2 kblocks, 30 rows, 32 cols]
RP = 32           # row pitch (28 cols + pad, %16 bytes)
KP = HP * RP      # per-kblock pitch = 960

_PROGRAM = None


def _build_program():
    nc = bacc.Bacc("TRN2", target_bir_lowering=False, debug=False,
                   num_devices=N_CORES)

    x_in = nc.dram_tensor("x", [IMGS, C, H, W], F32, kind="ExternalInput").ap()
    w1_in = nc.dram_tensor("w1", [C, C, 3, 3], F32, kind="ExternalInput").ap()
    w2_in = nc.dram_tensor("w2", [C, C, 3, 3], F32, kind="ExternalInput").ap()
    g1_in = nc.dram_tensor("gamma1", [C], F32, kind="ExternalInput").ap()
    b1_in = nc.dram_tensor("beta1", [C], F32, kind="ExternalInput").ap()
    g2_in = nc.dram_tensor("gamma2", [C], F32, kind="ExternalInput").ap()
    b2_in = nc.dram_tensor("beta2", [C], F32, kind="ExternalInput").ap()
    out_d = nc.dram_tensor("out", [IMGS, C, H, W], F32, kind="ExternalOutput").ap()

    groups = [list(range(N_CORES))]

    with tile.TileContext(nc) as tc:
        with (
            tc.tile_pool(name="consts", bufs=1) as p_const,
            tc.tile_pool(name="wstage", bufs=3) as p_wstage,
            tc.tile_pool(name="wsign", bufs=2) as p_wsign,
            tc.tile_pool(name="wt", bufs=2 * 9 * 2) as p_wt,
            tc.tile_pool(name="xp", bufs=IMGS * CB) as p_x,
            tc.tile_pool(name="apad", bufs=NPAD) as p_apad,
            tc.tile_pool(name="yz", bufs=IMGS * CB) as p_yz,
            tc.tile_pool(name="sq", bufs=2) as p_sq,
            tc.tile_pool(name="o1", bufs=8) as p_o1,
            tc.tile_pool(name="ps", bufs=8, space="PSUM") as p_ps,
            tc.tile_pool(name="dram", bufs=1, space="DRAM") as p_dram,
        ):
            # Warmup collective: absorbs the ~11us ncfw wake + first-mesh-op
            # overhead on stream 0 while conv1 runs, so the BN sync
            # AllReduces behave like warm ops (~10us instead of ~38us).
            ccw_i = p_dram.tile([128, 1], F32, name="ccw_i")
            ccw_o = p_dram.tile([128, 1], F32, name="ccw_o")
            zz = p_const.tile([128, 1], F32, name="zz")
            nc.vector.memset(zz, 0.0)
            nc.sync.dma_start(out=ccw_i, in_=zz)
            nc.gpsimd.collective_compute(
                "AllReduce", ALU.add, replica_groups=groups,
                ins=[ccw_i.opt()], outs=[ccw_o.opt()])

            # bf16 identity: weight transposes run at 1 cycle/row in bf16
            # (vs 2 for f32), halving their PE cost.
            ident = p_const.tile([128, 128], BF16, name="ident")
            make_identity(nc, ident)

            def dma_chunked(out_ap, in_ap, parts, engs):
                """Split a big DMA along the last free dim so the transfer
                spreads across several DMA engines; rotate the issuing
                queue over `engs` so the ~0.6us per-issue cost is
                parallelized across sequencers."""
                n = out_ap.shape[-1]
                step = (n + parts - 1) // parts
                for ci, a in enumerate(range(0, n, step)):
                    b = min(a + step, n)
                    engs[ci % len(engs)].dma_start(
                        out=out_ap[:, a:b], in_=in_ap[:, a:b])

            # gamma/beta as [128, 2] (col = channel block)
            def load_cvec(src, nm):
                t = p_const.tile([128, CB], F32, name=nm)
                nc.sync.dma_start(out=t, in_=src.rearrange("(b p) -> p b", p=128))
                return t

            g1t = load_cvec(g1_in, "g1t")
            b1t = load_cvec(b1_in, "b1t")
            rg1 = p_const.tile([128, CB], F32, name="rg1")
            nc.vector.reciprocal(rg1, g1t)
            bg1 = p_const.tile([128, CB], F32, name="bg1")
            nc.vector.tensor_mul(bg1, b1t, rg1)
            g2t = load_cvec(g2_in, "g2t")
            b2t = load_cvec(b2_in, "b2t")

            # per-channel stat accumulators, one column per (img, half)
            def stat_tiles(nm):
                return [p_const.tile([128, IMGS * 2], F32, name=f"{nm}{ob}")
                        for ob in range(CB)]

            st1s, st1q = stat_tiles("st1s"), stat_tiles("st1q")
            st2s, st2q = stat_tiles("st2s"), stat_tiles("st2q")

            # ---- padded fp8 activation buffers: border-only memsets ----
            # The binarize writes always cover [1:29, 1:29]; only the halo
            # (row 0, row 29, col 0, cols 29-31) must be zero, and it stays
            # zero when a buffer is recycled between conv1 and conv2 inputs.
            pad = [p_apad.tile([128, CB * KP], FP8, tag="apad", name=f"pad{i}")
                   for i in range(NPAD)]

            def memset_border(t):
                a4 = t.rearrange("p (k r c) -> p k r c", k=CB, r=HP)
                for b in range(CB):
                    nc.gpsimd.memset(a4[:, b, 0, :], 0.0)
                    nc.gpsimd.memset(a4[:, b, 29, :], 0.0)
                    nc.gpsimd.memset(a4[:, b, 1:29, 0:1], 0.0)
                    nc.gpsimd.memset(a4[:, b, 1:29, 29:32], 0.0)

            xsign = [pad[n] for n in range(IMGS)]
            b2a = [pad[(IMGS + n) % NPAD] for n in range(IMGS)]

            # ---- weight staging (DMA) and prep: one big Sign to bf16, then
            # cheap 1-cycle/row PE transposes, then PSUM->fp8 copies ----
            def stage_w(w_in, wi, ob):
                wst = p_wstage.tile([128, C * 9], F32, tag="wst",
                                    name=f"wst{wi}_{ob}")
                dma_chunked(
                    wst,
                    w_in[ob * 128:(ob + 1) * 128].rearrange(
                        "o i ky kx -> o (i ky kx)"),
                    parts=6, engs=[nc.sync, nc.scalar])
                return wst

            def prep_w(wst, wi, ob, wt):
                wsg = p_wsign.tile([128, C * 9], BF16, tag="wsg",
                                   name=f"wsg{wi}_{ob}")
                nc.scalar.activation(wsg, wst, AF.Sign)
                w3 = wsg.rearrange("p (i t) -> p i t", t=9)
                for tap in range(9):
                    t = p_wt.tile([128, CB * 128], FP8, tag="wt",
                                  name=f"wt{wi}_{tap}_{ob}")
                    wt[(tap, ob)] = t
                    for kb in range(CB):
                        ps = p_ps.tile([128, 128], BF16, tag="ps",
                                       name=f"pst{wi}_{ob}_{kb}_{tap}")
                        nc.tensor.transpose(
                            ps, w3[:, kb * 128:(kb + 1) * 128, tap], ident)
                        nc.scalar.activation(
                            t[:, kb * 128:(kb + 1) * 128], ps, AF.Copy)

            # ---- x: load raw f32 (kept for the residual), sign -> fp8 ----
            xt = [[None] * CB for _ in range(IMGS)]

            def load_x(n, parts):
                a4 = xsign[n].rearrange("p (k r c) -> p k r c", k=CB, r=HP)
                for b in range(CB):
                    xr = p_x.tile([128, PIX], F32, tag="xp", name=f"x_{n}_{b}")
                    dma_chunked(
                        xr,
                        x_in[n, b * 128:(b + 1) * 128].rearrange(
                            "c h w -> c (h w)"),
                        parts=parts,
                        engs=[nc.sync, nc.scalar] if n % 2 == 0
                        else [nc.scalar, nc.sync])
                    xt[n][b] = xr
                    nc.vector.tensor_scalar(
                        out=a4[:, b, 1:29, 1:29],
                        in0=xr.rearrange("p (h w) -> p h w", h=H),
                        scalar1=0.0, scalar2=0.5,
                        op0=ALU.is_ge, op1=ALU.subtract)

            # head: w1-ob0 staging leads, then the first image pair, then
            # w1-ob1 (needed at ~pair0-ob1), then the x stream, then w2.
            # GpSimd only runs the halo memsets here; the first two pad
            # buffers are zeroed before the img0/img1 binarize needs them.
            wt1, wt2 = {}, {}
            memset_border(pad[0])
            memset_border(pad[1])
            ws10 = stage_w(w1_in, 1, 0)
            load_x(0, parts=2)
            load_x(1, parts=2)
            ws11 = stage_w(w1_in, 1, 1)
            prep_w(ws10, 1, 0, wt1)
            for i in range(2, NPAD):
                memset_border(pad[i])
            for n in range(2, IMGS):
                load_x(n, parts=1)
            ws20 = stage_w(w2_in, 2, 0)
            ws21 = stage_w(w2_in, 2, 1)

            # ---- conv: 9 DoubleRow matmuls (K=256) per [128, 392] PSUM ----
            def emit_group(wt, act, evict, pair, ob):
                tiles = [(n, half)
                         for n in (2 * pair, 2 * pair + 1)
                         for half in range(2)]
                pss = {}
                for (n, half) in tiles:
                    pss[(n, half)] = p_ps.tile(
                        [128, HALF], F32, tag="ps",
                        name=f"ps_{ob}_{n}_{half}")
                for tap in range(9):
                    dy, dx = divmod(tap, 3)
                    w3 = wt[(tap, ob)].rearrange(
                        "p (k o) -> p k o", k=CB)
                    for (n, half) in tiles:
                        a4 = act[n].rearrange(
                            "p (k r c) -> p k r c", k=CB, r=HP)
                        rhs = a4[:, :, dy + half * 14: dy + half * 14 + 14,
                                 dx: dx + W]
                        nc.tensor.matmul(pss[(n, half)], w3, rhs,
                                         start=(tap == 0),
                                         stop=(tap == 8),
                                         perf_mode=DR)
                for (n, half) in tiles:
                    evict(n, ob, half, pss[(n, half)])

            # ---- conv1 eviction: copy PSUM->y1 with sum, square w/ sumsq ----
            y1 = [[None] * CB for _ in range(IMGS)]

            def evict1(n, ob, half, ps):
                if y1[n][ob] is None:
                    y1[n][ob] = p_yz.tile([128, PIX], F32, tag="yz",
                                          name=f"y1_{n}_{ob}")
                idx = n * 2 + half
                ysl = y1[n][ob][:, half * HALF:(half + 1) * HALF]
                nc.scalar.activation(ysl, ps, AF.Copy, scale=2.0,
                                     accum_out=st1s[ob][:, idx:idx + 1])
                sq = p_sq.tile([128, HALF], F32, tag="sq")
                nc.vector.scalar_tensor_tensor(
                    out=sq, in0=ysl, scalar=1.0, in1=ysl,
                    op0=ALU.mult, op1=ALU.mult,
                    accum_out=st1q[ob][:, idx:idx + 1])

            # conv1: image-major (pairs outer) relaxes the x-load deadlines;
            # w1-ob1 prep slots between the first two groups.
            emit_group(wt1, xsign, evict1, 0, 0)
            prep_w(ws11, 1, 1, wt1)
            emit_group(wt1, xsign, evict1, 0, 1)
            for pair in range(1, IMGS // 2):
                for ob in range(CB):
                    emit_group(wt1, xsign, evict1, pair, ob)

            # ---- BN1: AllReduce global sums, derive per-channel thresholds ----
            pk1 = p_const.tile([128, 2 * CB], F32, name="pk1")
            for ob in range(CB):
                nc.vector.tensor_reduce(out=pk1[:, 2 * ob:2 * ob + 1],
                                        in_=st1s[ob], axis=mybir.AxisListType.X,
                                        op=ALU.add)
                nc.vector.tensor_reduce(out=pk1[:, 2 * ob + 1:2 * ob + 2],
                                        in_=st1q[ob], axis=mybir.AxisListType.X,
                                        op=ALU.add)
            cc1i = p_dram.tile([128, 2 * CB], F32, name="cc1i")
            cc1o = p_dram.tile([128, 2 * CB], F32, name="cc1o")
            nc.sync.dma_start(out=cc1i, in_=pk1)
            nc.gpsimd.collective_compute(
                "AllReduce", ALU.add, replica_groups=groups,
                ins=[cc1i.opt()], outs=[cc1o.opt()])
            red1 = p_const.tile([128, 2 * CB], F32, name="red1")
            nc.sync.dma_start(out=red1, in_=cc1o)
            r3 = red1.rearrange("p (b k) -> p b k", k=2)

            # w2 prep fills the sync-BN wait on the PE
            prep_w(ws20, 2, 0, wt2)
            prep_w(ws21, 2, 1, wt2)

            m1 = p_const.tile([128, CB], F32, name="m1")
            nc.vector.tensor_scalar(out=m1, in0=r3[:, :, 0], scalar1=1.0 / NT,
                                    scalar2=None, op0=ALU.mult)
            e1 = p_const.tile([128, CB], F32, name="e1")
            nc.vector.tensor_scalar(out=e1, in0=r3[:, :, 1], scalar1=1.0 / NT,
                                    scalar2=None, op0=ALU.mult)
            mm1 = p_const.tile([128, CB], F32, name="mm1")
            nc.vector.tensor_mul(mm1, m1, m1)
            v1 = p_const.tile([128, CB], F32, name="v1")
            nc.vector.tensor_sub(v1, e1, mm1)
            v1e = p_const.tile([128, CB], F32, name="v1e")
            nc.vector.tensor_scalar(out=v1e, in0=v1, scalar1=EPS, scalar2=None,
                                    op0=ALU.add)
            sd1 = p_const.tile([128, CB], F32, name="sd1")
            nc.scalar.activation(sd1, v1e, AF.Sqrt)
            tb1 = p_const.tile([128, CB], F32, name="tb1")
            nc.vector.tensor_mul(tb1, bg1, sd1)
            thr1 = p_const.tile([128, CB], F32, name="thr1")
            nc.vector.tensor_sub(thr1, m1, tb1)

            # ---- binarize(BN1(y1)) == is_ge(y1, thr) - 0.5 (padded fp8) ----
            for n in range(IMGS):
                a4 = b2a[n].rearrange("p (k r c) -> p k r c", k=CB, r=HP)
                for b in range(CB):
                    nc.vector.tensor_scalar(
                        out=a4[:, b, 1:29, 1:29],
                        in0=y1[n][b].rearrange("p (h w) -> p h w", h=H),
                        scalar1=thr1[:, b:b + 1], scalar2=0.5,
                        op0=ALU.is_ge, op1=ALU.subtract)

            # ---- conv2 eviction: z = 2*psum + x (fused sum), square ----
            z = [[None] * CB for _ in range(IMGS)]

            def evict2(n, ob, half, ps):
                if z[n][ob] is None:
                    z[n][ob] = p_yz.tile([128, PIX], F32, tag="yz",
                                         name=f"z_{n}_{ob}")
                idx = n * 2 + half
                zsl = z[n][ob][:, half * HALF:(half + 1) * HALF]
                nc.vector.scalar_tensor_tensor(
                    out=zsl, in0=ps, scalar=2.0,
                    in1=xt[n][ob][:, half * HALF:(half + 1) * HALF],
                    op0=ALU.mult, op1=ALU.add,
                    accum_out=st2s[ob][:, idx:idx + 1])
                sq = p_sq.tile([128, HALF], F32, tag="sq")
                nc.scalar.activation(sq, zsl, AF.Square,
                                     accum_out=st2q[ob][:, idx:idx + 1])

            # ---- BN2 per output block: reduce, AllReduce, affine params ----
            def bn2_sync(ob):
                pk = p_const.tile([128, 2], F32, name=f"pk2_{ob}")
                nc.vector.tensor_reduce(out=pk[:, 0:1], in_=st2s[ob],
                                        axis=mybir.AxisListType.X, op=ALU.add)
                nc.vector.tensor_reduce(out=pk[:, 1:2], in_=st2q[ob],
                                        axis=mybir.AxisListType.X, op=ALU.add)
                cci = p_dram.tile([128, 2], F32, name=f"cc2i_{ob}")
                cco = p_dram.tile([128, 2], F32, name=f"cc2o_{ob}")
                nc.sync.dma_start(out=cci, in_=pk)
                nc.gpsimd.collective_compute(
                    "AllReduce", ALU.add, replica_groups=groups,
                    ins=[cci.opt()], outs=[cco.opt()])
                red = p_const.tile([128, 2], F32, name=f"red2_{ob}")
                nc.sync.dma_start(out=red, in_=cco)
                m2 = p_const.tile([128, 1], F32, name=f"m2_{ob}")
                nc.vector.tensor_scalar(out=m2, in0=red[:, 0:1],
                                        scalar1=1.0 / NT, scalar2=None,
                                        op0=ALU.mult)
                e2 = p_const.tile([128, 1], F32, name=f"e2_{ob}")
                nc.vector.tensor_scalar(out=e2, in0=red[:, 1:2],
                                        scalar1=1.0 / NT, scalar2=None,
                                        op0=ALU.mult)
                mm2 = p_const.tile([128, 1], F32, name=f"mm2_{ob}")
                nc.vector.tensor_mul(mm2, m2, m2)
                v2 = p_const.tile([128, 1], F32, name=f"v2_{ob}")
                nc.vector.tensor_sub(v2, e2, mm2)
                v2f = p_const.tile([128, 1], F32, name=f"v2f_{ob}")
                nc.vector.tensor_scalar(out=v2f, in0=v2, scalar1=EPS,
                                        scalar2=None, op0=ALU.add)
                rc2 = p_const.tile([128, 1], F32, name=f"rc2_{ob}")
                nc.vector.reciprocal(rc2, v2f)
                rstd = p_const.tile([128, 1], F32, name=f"rstd_{ob}")
                nc.scalar.activation(rstd, rc2, AF.Sqrt)
                fs = p_const.tile([128, 1], F32, name=f"fs_{ob}")
                nc.vector.tensor_mul(fs, g2t[:, ob:ob + 1], rstd)
                msc = p_const.tile([128, 1], F32, name=f"msc_{ob}")
                nc.vector.tensor_mul(msc, m2, fs)
                fb = p_const.tile([128, 1], F32, name=f"fb_{ob}")
                nc.vector.tensor_sub(fb, b2t[:, ob:ob + 1], msc)
                return fs, fb

            # ---- final: clip(z * fscale + fbias) -> DRAM ----
            def out_tile(n, ob, fs, fb, aff_eng, clamp_eng, dma_engs):
                o1 = p_o1.tile([128, PIX], F32, tag="o1")
                if aff_eng is nc.scalar:
                    nc.scalar.activation(o1, z[n][ob], AF.Identity,
                                         bias=fb, scale=fs)
                else:
                    aff_eng.tensor_scalar(
                        out=o1, in0=z[n][ob], scalar1=fs, scalar2=fb,
                        op0=ALU.mult, op1=ALU.add)
                clamp_eng.tensor_scalar(out=o1, in0=o1, scalar1=-1.0,
                                        scalar2=1.0, op0=ALU.max, op1=ALU.min)
                dma_chunked(
                    out_d[n, ob * 128:(ob + 1) * 128].rearrange(
                        "c h w -> c (h w)"),
                    o1, parts=2, engs=dma_engs)

            # conv2-ob0, then its BN sync; the ob0 output phase rides on
            # GpSimd (affine) + Vector (clamp; GpSimd MAX/MIN is ~10x slow)
            # + the sync DMA queue underneath conv2-ob1's PE time.
            for pair in range(IMGS // 2):
                emit_group(wt2, b2a, evict2, pair, 0)
            fs0, fb0 = bn2_sync(0)

            emit_group(wt2, b2a, evict2, 0, 1)
            emit_group(wt2, b2a, evict2, 1, 1)
            emit_group(wt2, b2a, evict2, 2, 1)
            for n in range(4):
                out_tile(n, 0, fs0, fb0, nc.gpsimd, nc.vector, [nc.sync])
            emit_group(wt2, b2a, evict2, 3, 1)
            for n in range(4, IMGS):
                out_tile(n, 0, fs0, fb0, nc.gpsimd, nc.vector, [nc.sync])

            fs1, fb1 = bn2_sync(1)
            for n in range(IMGS):
                aff = nc.scalar if n % 2 == 0 else nc.gpsimd
                dmae = [[nc.sync, nc.gpsimd], [nc.scalar, nc.sync]][n % 2]
                out_tile(n, 1, fs1, fb1, aff, nc.vector, dmae)

    nc.compile()
    return nc


def _get_program():
    global _PROGRAM
    if _PROGRAM is None:
        _PROGRAM = _build_program()
    return _PROGRAM


def run_sharded(inputs, **spmd_kwargs):
    """Shard inputs across 8 cores, run, and gather. Returns (out, results)."""
    nc = _get_program()
    x = np.ascontiguousarray(np.asarray(inputs["x"], dtype=np.float32))
    base = {
        k: np.ascontiguousarray(np.asarray(inputs[k], dtype=np.float32))
        for k in ("w1", "w2", "gamma1", "beta1", "gamma2", "beta2")
    }
    shards = np.split(x, N_CORES, axis=0)
    in_maps = [{"x": shards[i], **base} for i in range(N_CORES)]
    res = run_bass_kernel_spmd(nc, in_maps, core_ids=list(range(N_CORES)),
                               **spmd_kwargs)
    out = np.concatenate([res.results[i]["out"] for i in range(N_CORES)],
                         axis=0).astype(np.float32)
    return out, res


def kernel(**inputs):
    out, _ = run_sharded(inputs)
    return out


# revision 17
# speedup vs baseline: 1.7546x; 1.0478x over previous
"""Trainium2 Bass kernel for a binarized-conv BasicBlock (dense_cnn).

Computation (matches the reference nn.Module):
    out = clip(BN2(conv3x3(binarize(clip(BN1(conv3x3(binarize(x), binarize(w1))))),
                  binarize(w2)) + x))
with training-mode (batch-stats) BN over the full 64-image batch.

Strategy:
  - Data-parallel over batch: 8 images per core on 8 NeuronCores.
  - Binarized 3x3 conv as 9 accumulating DoubleRow fp8 PE matmuls (K=256)
    per [128, 392] output tile over zero-padded [128, 2, 30, 32] fp8
    activation tiles; +-1/+-0.5 values in fp8 are exact, PSUM is fp32.
  - BN1 + hardtanh + binarize collapses to a per-channel threshold compare;
    sync-BN via a single [128, 4] AllReduce, its latency bridged by the w2
    weight-prep transposes on the otherwise-idle PE.
  - conv2 runs output-block-major so BN2's sync splits per block: AR2a and
    the whole ob0 output phase (affine+clamp+DMA) hide under conv2-ob1's PE
    time; only AR2b plus the ob1 output burst remain in the tail.
  - Padded activation buffers get border-only memsets and are recycled
    between conv1 and conv2 inputs (interior is always fully rewritten).
"""

import os
import sys

import numpy as np


def _ensure_paths():
    for p in ("/opt/trn_rl_repo", "/root/.axon_site/_ro/trn_rl_repo"):
        if p not in sys.path and os.path.isdir(p):
            sys.path.append(p)


try:
    from concourse import bacc, mybir, tile  # noqa: F401
except ImportError:
    _ensure_paths()
    from concourse import bacc, mybir, tile  # noqa: F401

from concourse.bass_utils import run_bass_kernel_spmd
from concourse.masks import make_identity

N_CORES = 8
IMGS = 8          # images per core (64 / 8)
C = 256
CB = 2            # channel blocks of 128
H = W = 28
HP = WP = 30      # zero-padded spatial
PIX = H * W       # 784
HALF = PIX // 2   # 392 (one PSUM bank of fp32)
NT = 64 * PIX     # BN count over the GLOBAL batch (N*H*W)
EPS = 1e-5
NPAD = IMGS + 2   # physical padded-activation buffers (recycled)

F32 = mybir.dt.float32
BF16 = mybir.dt.bfloat16
FP8 = mybir.dt.float8e4
AF = mybir.ActivationFunctionType
ALU = mybir.AluOpType
DR = mybir.MatmulPerfMode.DoubleRow

# padded fp8 activation layout: [128, 2 kblocks, 30 rows, 32 cols]
RP = 32           # row pitch (28 cols + pad, %16 bytes)
KP = HP * RP      # per-kblock pitch = 960

_PROGRAM = None


def _build_program():
    nc = bacc.Bacc("TRN2", target_bir_lowering=False, debug=False,
                   num_devices=N_CORES)

    x_in = nc.dram_tensor("x", [IMGS, C, H, W], F32, kind="ExternalInput").ap()
    w1_in = nc.dram_tensor("w1", [C, C, 3, 3], F32, kind="ExternalInput").ap()
    w2_in = nc.dram_tensor("w2", [C, C, 3, 3], F32, kind="ExternalInput").ap()
    g1_in = nc.dram_tensor("gamma1", [C], F32, kind="ExternalInput").ap()
    b1_in = nc.dram_tensor("beta1", [C], F32, kind="ExternalInput").ap()
    g2_in = nc.dram_tensor("gamma2", [C], F32, kind="ExternalInput").ap()
    b2_in = nc.dram_tensor("beta2", [C], F32, kind="ExternalInput").ap()
    out_d = nc.dram_tensor("out", [IMGS, C, H, W], F32, kind="ExternalOutput").ap()

    groups = [list(range(N_CORES))]

    with tile.TileContext(nc) as tc:
        with (
            tc.tile_pool(name="consts", bufs=1) as p_const,
            tc.tile_pool(name="wstage", bufs=3) as p_wstage,
            tc.tile_pool(name="wsign", bufs=2) as p_wsign,
            tc.tile_pool(name="wt", bufs=2 * 9 * 2) as p_wt,
            tc.tile_pool(name="xp", bufs=IMGS * CB) as p_x,
            tc.tile_pool(name="apad", bufs=NPAD) as p_apad,
            tc.tile_pool(name="yz", bufs=IMGS * CB) as p_yz,
            tc.tile_pool(name="sq", bufs=2) as p_sq,
            tc.tile_pool(name="o1", bufs=8) as p_o1,
            tc.tile_pool(name="ps", bufs=8, space="PSUM") as p_ps,
            tc.tile_pool(name="dram", bufs=1, space="DRAM") as p_dram,
        ):
            # Warmup collective: absorbs the ~11us ncfw wake + first-mesh-op
            # overhead on stream 0 while conv1 runs, so the BN sync
            # AllReduces behave like warm ops (~10us instead of ~38us).
            ccw_i = p_dram.tile([128, 1], F32, name="ccw_i")
            ccw_o = p_dram.tile([128, 1], F32, name="ccw_o")
            zz = p_const.tile([128, 1], F32, name="zz")
            nc.vector.memset(zz, 0.0)
            nc.sync.dma_start(out=ccw_i, in_=zz)
            nc.gpsimd.collective_compute(
                "AllReduce", ALU.add, replica_groups=groups,
                ins=[ccw_i.opt()], outs=[ccw_o.opt()])

            # bf16 identity: weight transposes run at 1 cycle/row in bf16
            # (vs 2 for f32), halving their PE cost.
            ident = p_const.tile([128, 128], BF16, name="ident")
            make_identity(nc, ident)

            def dma_chunked(out_ap, in_ap, parts, engs):
                """Split a big DMA along the last free dim so the transfer
                spreads across several DMA engines; rotate the issuing
                queue over `engs` so the ~0.6us per-issue cost is
                parallelized across sequencers."""
                n = out_ap.shape[-1]
                step = (n + parts - 1) // parts
                for ci, a in enumerate(range(0, n, step)):
                    b = min(a + step, n)
                    engs[ci % len(engs)].dma_start(
                        out=out_ap[:, a:b], in_=in_ap[:, a:b])

            # gamma/beta as [128, 2] (col = channel block)
            def load_cvec(src, nm):
                t = p_const.tile([128, CB], F32, name=nm)
                nc.sync.dma_start(out=t, in_=src.rearrange("(b p) -> p b", p=128))
                return t

            g1t = load_cvec(g1_in, "g1t")
            b1t = load_cvec(b1_in, "b1t")
            rg1 = p_const.tile([128, CB], F32, name="rg1")
            nc.vector.reciprocal(rg1, g1t)
            bg1 = p_const.tile([128, CB], F32, name="bg1")
            nc.vector.tensor_mul(bg1, b1t, rg1)
            g2t = load_cvec(g2_in, "g2t")
            b2t = load_cvec(b2_in, "b2t")

            # per-channel stat accumulators, one column per (img, half)
            def stat_tiles(nm):
                return [p_const.tile([128, IMGS * 2], F32, name=f"{nm}{ob}")
                        for ob in range(CB)]

            st1s, st1q = stat_tiles("st1s"), stat_tiles("st1q")
            st2s, st2q = stat_tiles("st2s"), stat_tiles("st2q")

            # ---- padded fp8 activation buffers: border-only memsets ----
            # The binarize writes always cover [1:29, 1:29]; only the halo
            # (row 0, row 29, col 0, cols 29-31) must be zero, and it stays
            # zero when a buffer is recycled between conv1 and conv2 inputs.
            pad = [p_apad.tile([128, CB * KP], FP8, tag="apad", name=f"pad{i}")
                   for i in range(NPAD)]

            def memset_border(t):
                a4 = t.rearrange("p (k r c) -> p k r c", k=CB, r=HP)
                for b in range(CB):
                    nc.gpsimd.memset(a4[:, b, 0, :], 0.0)
                    nc.gpsimd.memset(a4[:, b, 29, :], 0.0)
                    nc.gpsimd.memset(a4[:, b, 1:29, 0:1], 0.0)
                    nc.gpsimd.memset(a4[:, b, 1:29, 29:32], 0.0)

            xsign = [pad[n] for n in range(IMGS)]
            b2a = [pad[(IMGS + n) % NPAD] for n in range(IMGS)]

            # ---- weight staging (DMA) and prep: one big Sign to bf16, then
            # cheap 1-cycle/row PE transposes, then PSUM->fp8 copies ----
            def stage_w(w_in, wi, ob):
                wst = p_wstage.tile([128, C * 9], F32, tag="wst",
                                    name=f"wst{wi}_{ob}")
                dma_chunked(
                    wst,
                    w_in[ob * 128:(ob + 1) * 128].rearrange(
                        "o i ky kx -> o (i ky kx)"),
                    parts=6, engs=[nc.sync, nc.scalar])
                return wst

            def prep_w(wst, wi, ob, wt):
                wsg = p_wsign.tile([128, C * 9], BF16, tag="wsg",
                                   name=f"wsg{wi}_{ob}")
                nc.scalar.activation(wsg, wst, AF.Sign)
                w3 = wsg.rearrange("p (i t) -> p i t", t=9)
                for tap in range(9):
                    t = p_wt.tile([128, CB * 128], FP8, tag="wt",
                                  name=f"wt{wi}_{tap}_{ob}")
                    wt[(tap, ob)] = t
                    for kb in range(CB):
                        ps = p_ps.tile([128, 128], BF16, tag="ps",
                                       name=f"pst{wi}_{ob}_{kb}_{tap}")
                        nc.tensor.transpose(
                            ps, w3[:, kb * 128:(kb + 1) * 128, tap], ident)
                        nc.scalar.activation(
                            t[:, kb * 128:(kb + 1) * 128], ps, AF.Copy)

            # ---- x: load raw f32 (kept for the residual), sign -> fp8 ----
            xt = [[None] * CB for _ in range(IMGS)]

            def load_x(n, parts):
                a4 = xsign[n].rearrange("p (k r c) -> p k r c", k=CB, r=HP)
                for b in range(CB):
                    xr = p_x.tile([128, PIX], F32, tag="xp", name=f"x_{n}_{b}")
                    dma_chunked(
                        xr,
                        x_in[n, b * 128:(b + 1) * 128].rearrange(
                            "c h w -> c (h w)"),
                        parts=parts,
                        engs=[nc.sync, nc.scalar] if n % 2 == 0
                        else [nc.scalar, nc.sync])
                    xt[n][b] = xr
                    nc.vector.tensor_scalar(
                        out=a4[:, b, 1:29, 1:29],
                        in0=xr.rearrange("p (h w) -> p h w", h=H),
                        scalar1=0.0, scalar2=0.5,
                        op0=ALU.is_ge, op1=ALU.subtract)

            # head: w1-ob0 staging + the first image pair lead the DMA
            # queues (nothing else competes until they are in flight), then
            # w1-ob1 and the x stream; w2 staging is deferred until after
            # conv1 is emitted so its transfers ride behind the x stream.
            # GpSimd only runs the halo memsets here; the first two pad
            # buffers are zeroed before the img0/img1 binarize needs them.
            wt1, wt2 = {}, {}
            memset_border(pad[0])
            memset_border(pad[1])
            ws10 = stage_w(w1_in, 1, 0)
            load_x(0, parts=2)
            load_x(1, parts=2)
            prep_w(ws10, 1, 0, wt1)
            ws11 = stage_w(w1_in, 1, 1)
            for n in range(2, IMGS):
                load_x(n, parts=1)
            for i in range(2, NPAD):
                memset_border(pad[i])

            # ---- conv: 9 DoubleRow matmuls (K=256) per [128, 392] PSUM ----
            def emit_group(wt, act, evict, pair, ob, tiles=None):
                if tiles is None:
                    tiles = [(n, half)
                             for n in (2 * pair, 2 * pair + 1)
                             for half in range(2)]
                pss = {}
                for (n, half) in tiles:
                    pss[(n, half)] = p_ps.tile(
                        [128, HALF], F32, tag="ps",
                        name=f"ps_{ob}_{n}_{half}")
                for tap in range(9):
                    dy, dx = divmod(tap, 3)
                    w3 = wt[(tap, ob)].rearrange(
                        "p (k o) -> p k o", k=CB)
                    for (n, half) in tiles:
                        a4 = act[n].rearrange(
                            "p (k r c) -> p k r c", k=CB, r=HP)
                        rhs = a4[:, :, dy + half * 14: dy + half * 14 + 14,
                                 dx: dx + W]
                        nc.tensor.matmul(pss[(n, half)], w3, rhs,
                                         start=(tap == 0),
                                         stop=(tap == 8),
                                         perf_mode=DR)
                for (n, half) in tiles:
                    evict(n, ob, half, pss[(n, half)])

            # ---- conv1 eviction: copy PSUM->y1 with sum, square w/ sumsq ----
            y1 = [[None] * CB for _ in range(IMGS)]

            def evict1(n, ob, half, ps):
                if y1[n][ob] is None:
                    y1[n][ob] = p_yz.tile([128, PIX], F32, tag="yz",
                                          name=f"y1_{n}_{ob}")
                idx = n * 2 + half
                ysl = y1[n][ob][:, half * HALF:(half + 1) * HALF]
                nc.scalar.activation(ysl, ps, AF.Copy, scale=2.0,
                                     accum_out=st1s[ob][:, idx:idx + 1])
                sq = p_sq.tile([128, HALF], F32, tag="sq")
                nc.vector.scalar_tensor_tensor(
                    out=sq, in0=ysl, scalar=1.0, in1=ysl,
                    op0=ALU.mult, op1=ALU.mult,
                    accum_out=st1q[ob][:, idx:idx + 1])

            # conv1: image-major (pairs outer) relaxes the x-load deadlines.
            # The first two groups are single-image so the very first
            # matmuls need only img0; w1-ob1 prep slots in between.
            emit_group(wt1, xsign, evict1, 0, 0, tiles=[(0, 0), (0, 1)])
            emit_group(wt1, xsign, evict1, 0, 0, tiles=[(1, 0), (1, 1)])
            prep_w(ws11, 1, 1, wt1)
            emit_group(wt1, xsign, evict1, 0, 1)
            for pair in range(1, IMGS // 2):
                for ob in range(CB):
                    emit_group(wt1, xsign, evict1, pair, ob)

            # w2 staging: issued here so the transfers queue up behind the
            # x stream and land well before the BN1 sync window.
            ws20 = stage_w(w2_in, 2, 0)
            ws21 = stage_w(w2_in, 2, 1)

            # ---- BN1: AllReduce global sums, derive per-channel thresholds ----
            pk1 = p_const.tile([128, 2 * CB], F32, name="pk1")
            for ob in range(CB):
                nc.vector.tensor_reduce(out=pk1[:, 2 * ob:2 * ob + 1],
                                        in_=st1s[ob], axis=mybir.AxisListType.X,
                                        op=ALU.add)
                nc.vector.tensor_reduce(out=pk1[:, 2 * ob + 1:2 * ob + 2],
                                        in_=st1q[ob], axis=mybir.AxisListType.X,
                                        op=ALU.add)
            cc1i = p_dram.tile([128, 2 * CB], F32, name="cc1i")
            cc1o = p_dram.tile([128, 2 * CB], F32, name="cc1o")
            nc.sync.dma_start(out=cc1i, in_=pk1)
            nc.gpsimd.collective_compute(
                "AllReduce", ALU.add, replica_groups=groups,
                ins=[cc1i.opt()], outs=[cc1o.opt()])
            red1 = p_const.tile([128, 2 * CB], F32, name="red1")
            nc.sync.dma_start(out=red1, in_=cc1o)
            r3 = red1.rearrange("p (b k) -> p b k", k=2)

            # w2 prep fills the sync-BN wait on the PE
            prep_w(ws20, 2, 0, wt2)
            prep_w(ws21, 2, 1, wt2)

            m1 = p_const.tile([128, CB], F32, name="m1")
            nc.vector.tensor_scalar(out=m1, in0=r3[:, :, 0], scalar1=1.0 / NT,
                                    scalar2=None, op0=ALU.mult)
            e1 = p_const.tile([128, CB], F32, name="e1")
            nc.vector.tensor_scalar(out=e1, in0=r3[:, :, 1], scalar1=1.0 / NT,
                                    scalar2=None, op0=ALU.mult)
            mm1 = p_const.tile([128, CB], F32, name="mm1")
            nc.vector.tensor_mul(mm1, m1, m1)
            v1 = p_const.tile([128, CB], F32, name="v1")
            nc.vector.tensor_sub(v1, e1, mm1)
            v1e = p_const.tile([128, CB], F32, name="v1e")
            nc.vector.tensor_scalar(out=v1e, in0=v1, scalar1=EPS, scalar2=None,
                                    op0=ALU.add)
            sd1 = p_const.tile([128, CB], F32, name="sd1")
            nc.scalar.activation(sd1, v1e, AF.Sqrt)
            tb1 = p_const.tile([128, CB], F32, name="tb1")
            nc.vector.tensor_mul(tb1, bg1, sd1)
            thr1 = p_const.tile([128, CB], F32, name="thr1")
            nc.vector.tensor_sub(thr1, m1, tb1)

            # ---- binarize(BN1(y1)) == is_ge(y1, thr) - 0.5 (padded fp8) ----
            for n in range(IMGS):
                a4 = b2a[n].rearrange("p (k r c) -> p k r c", k=CB, r=HP)
                for b in range(CB):
                    nc.vector.tensor_scalar(
                        out=a4[:, b, 1:29, 1:29],
                        in0=y1[n][b].rearrange("p (h w) -> p h w", h=H),
                        scalar1=thr1[:, b:b + 1], scalar2=0.5,
                        op0=ALU.is_ge, op1=ALU.subtract)

            # ---- conv2 eviction: z = 2*psum + x (fused sum), square ----
            z = [[None] * CB for _ in range(IMGS)]

            def evict2(n, ob, half, ps):
                if z[n][ob] is None:
                    z[n][ob] = p_yz.tile([128, PIX], F32, tag="yz",
                                         name=f"z_{n}_{ob}")
                idx = n * 2 + half
                zsl = z[n][ob][:, half * HALF:(half + 1) * HALF]
                nc.vector.scalar_tensor_tensor(
                    out=zsl, in0=ps, scalar=2.0,
                    in1=xt[n][ob][:, half * HALF:(half + 1) * HALF],
                    op0=ALU.mult, op1=ALU.add,
                    accum_out=st2s[ob][:, idx:idx + 1])
                sq = p_sq.tile([128, HALF], F32, tag="sq")
                nc.scalar.activation(sq, zsl, AF.Square,
                                     accum_out=st2q[ob][:, idx:idx + 1])

            # ---- BN2 per output block: reduce, AllReduce, affine params.
            # The post-AR chain runs on GpSimd (idle then) + one Scalar
            # Rsqrt so the busy Vector/Sync queues never gate fs/fb. ----
            def bn2_sync(ob):
                pk = p_const.tile([128, 2], F32, name=f"pk2_{ob}")
                nc.vector.tensor_reduce(out=pk[:, 0:1], in_=st2s[ob],
                                        axis=mybir.AxisListType.X, op=ALU.add)
                nc.vector.tensor_reduce(out=pk[:, 1:2], in_=st2q[ob],
                                        axis=mybir.AxisListType.X, op=ALU.add)
                cci = p_dram.tile([128, 2], F32, name=f"cc2i_{ob}")
                cco = p_dram.tile([128, 2], F32, name=f"cc2o_{ob}")
                nc.scalar.dma_start(out=cci, in_=pk)
                nc.gpsimd.collective_compute(
                    "AllReduce", ALU.add, replica_groups=groups,
                    ins=[cci.opt()], outs=[cco.opt()])
                red = p_const.tile([128, 2], F32, name=f"red2_{ob}")
                nc.scalar.dma_start(out=red, in_=cco)
                m2 = p_const.tile([128, 1], F32, name=f"m2_{ob}")
                nc.gpsimd.tensor_scalar(out=m2, in0=red[:, 0:1],
                                        scalar1=1.0 / NT, scalar2=None,
                                        op0=ALU.mult)
                mm2 = p_const.tile([128, 1], F32, name=f"mm2_{ob}")
                nc.gpsimd.tensor_scalar(out=mm2, in0=red[:, 0:1],
                                        scalar1=red[:, 0:1],
                                        scalar2=1.0 / (NT * NT),
                                        op0=ALU.mult, op1=ALU.mult)
                v2f = p_const.tile([128, 1], F32, name=f"v2f_{ob}")
                nc.gpsimd.tensor_scalar(out=v2f, in0=red[:, 1:2],
                                        scalar1=1.0 / NT, scalar2=EPS,
                                        op0=ALU.mult, op1=ALU.add)
                v2 = p_const.tile([128, 1], F32, name=f"v2_{ob}")
                nc.gpsimd.tensor_sub(v2, v2f, mm2)
                rc2 = p_const.tile([128, 1], F32, name=f"rc2_{ob}")
                nc.vector.reciprocal(rc2, v2)
                rstd = p_const.tile([128, 1], F32, name=f"rstd_{ob}")
                nc.scalar.activation(rstd, rc2, AF.Sqrt)
                fs = p_const.tile([128, 1], F32, name=f"fs_{ob}")
                nc.gpsimd.tensor_mul(fs, g2t[:, ob:ob + 1], rstd)
                msc = p_const.tile([128, 1], F32, name=f"msc_{ob}")
                nc.gpsimd.tensor_mul(msc, m2, fs)
                fb = p_const.tile([128, 1], F32, name=f"fb_{ob}")
                nc.gpsimd.tensor_sub(fb, b2t[:, ob:ob + 1], msc)
                return fs, fb

            # ---- final: clip(z * fscale + fbias) -> DRAM ----
            def out_tile(n, ob, fs, fb, aff_eng, clamp_eng, dma_engs):
                o1 = p_o1.tile([128, PIX], F32, tag="o1")
                if aff_eng is nc.scalar:
                    nc.scalar.activation(o1, z[n][ob], AF.Identity,
                                         bias=fb, scale=fs)
                else:
                    aff_eng.tensor_scalar(
                        out=o1, in0=z[n][ob], scalar1=fs, scalar2=fb,
                        op0=ALU.mult, op1=ALU.add)
                clamp_eng.tensor_scalar(out=o1, in0=o1, scalar1=-1.0,
                                        scalar2=1.0, op0=ALU.max, op1=ALU.min)
                dma_chunked(
                    out_d[n, ob * 128:(ob + 1) * 128].rearrange(
                        "c h w -> c (h w)"),
                    o1, parts=2, engs=dma_engs)

            # conv2-ob0, then its BN sync; the ob0 output phase rides on
            # GpSimd (affine) + Vector (clamp; GpSimd MAX/MIN is ~10x slow)
            # + the sync DMA queue underneath conv2-ob1's PE time.
            for pair in range(IMGS // 2):
                emit_group(wt2, b2a, evict2, pair, 0)
            fs0, fb0 = bn2_sync(0)

            emit_group(wt2, b2a, evict2, 0, 1)
            emit_group(wt2, b2a, evict2, 1, 1)
            for n in range(2):
                out_tile(n, 0, fs0, fb0, nc.gpsimd, nc.vector,
                         [nc.sync, nc.scalar])
            emit_group(wt2, b2a, evict2, 2, 1)
            for n in range(2, 4):
                out_tile(n, 0, fs0, fb0, nc.gpsimd, nc.vector,
                         [nc.sync, nc.scalar])
            emit_group(wt2, b2a, evict2, 3, 1)
            for n in range(4, IMGS):
                out_tile(n, 0, fs0, fb0, nc.gpsimd, nc.vector,
                         [nc.sync, nc.scalar])

            fs1, fb1 = bn2_sync(1)
            for n in range(IMGS):
                aff = nc.scalar if n % 2 == 0 else nc.gpsimd
                dmae = [[nc.sync, nc.scalar], [nc.scalar, nc.sync]][n % 2]
                out_tile(n, 1, fs1, fb1, aff, nc.vector, dmae)

    nc.compile()
    return nc


def _get_program():
    global _PROGRAM
    if _PROGRAM is None:
        _PROGRAM = _build_program()
    return _PROGRAM


def run_sharded(inputs, **spmd_kwargs):
    """Shard inputs across 8 cores, run, and gather. Returns (out, results)."""
    nc = _get_program()
    x = np.ascontiguousarray(np.asarray(inputs["x"], dtype=np.float32))
    base = {
        k: np.ascontiguousarray(np.asarray(inputs[k], dtype=np.float32))
        for k in ("w1", "w2", "gamma1", "beta1", "gamma2", "beta2")
    }
    shards = np.split(x, N_CORES, axis=0)
    in_maps = [{"x": shards[i], **base} for i in range(N_CORES)]
    res = run_bass_kernel_spmd(nc, in_maps, core_ids=list(range(N_CORES)),
                               **spmd_kwargs)
    out = np.concatenate([res.results[i]["out"] for i in range(N_CORES)],
                         axis=0).astype(np.float32)
    return out, res


def kernel(**inputs):
    out, _ = run_sharded(inputs)
    return out
